# revision 23
# baseline (speedup 1.0000x reference)
# Multi-headed attention (B=2, A=6, S=1024, E=256, d_model=512, H=8, DK=64)
# distributed over 8 NeuronCores.
#
# Decomposition: the 12 (batch, agent) pairs are each split into two
# "quad-tasks" of 4 heads (d_model halves), giving 24 tasks; each core runs
# 3 tasks (perfect balance, no duplicated FLOPs: QKV projections split
# cleanly along the head dim, the output projection's head contraction is
# summed on the host).
#
# Per-task device pipeline (no on-device transposes anywhere):
#   inputs arrive host-pre-transposed as x^T [E, S].
#   QT = Wq_t^T @ q^T   [F=256, S]      (lhsT = Wq_t, rhs = q^T)
#   KT = Wk_t^T @ k^T   [F=256, S]
#   V  = (v^T)^T @ Wv_t [S, F]          (lhsT = v^T slice, rhs = Wv_t)
#   per head h (64 rows of QT/KT):
#     scoresT[k, q] = K_h @ Q_h^T       (lhsT = KT_h slice, rhs = QT_h slice)
#     pT = exp(scoresT / 8)             (ONE [128,1024] ACT instr per k-tile;
#                                        no max subtraction -- scores O(1))
#     xT[65, S]  = [V_h | 1]^T @ pT     (row 64 = softmax denominators)
#     outT_h = Wo_h^T @ xT[0:64]        (RAW, unnormalized)
#   ship outT_h and the denominators; the host divides (normalization
#   commutes with the per-head linear) and sums heads.
#
# Schedule: a flat slot pipeline balanced against the ACT engine's exp
# floor (~1.0 us per [128,1024] tile).  Each slot (t,h,m) emits, in PE
# program order: the scores pair for (h,m), then the AV pair for the
# PREVIOUS slot (software-pipelined by one so the exp latency is hidden),
# then at most ~1 "foreign" matmul unit (output projection of the previous
# head at m=1..4, next-task QK/V projection units at m=0/5/6/7).  ACT does
# exps only; all PSUM evacuations run on DVE + Pool(gpsimd).
import numpy as np

import concourse.bass as bass
from concourse import bacc
import concourse.mybir as mybir
from concourse.tile import TileContext
from concourse.bass_utils import run_bass_kernel_spmd
from contextlib import ExitStack

B, A, S, E = 2, 6, 1024, 256
DMODEL, H, DK = 512, 8, 64
F = 256                 # per-task projection width (4 heads x 64)
OUTD = 256              # output dim (q_dim)
NT = 3                  # tasks per core
NCORES = 8
P = 128
NPAIR = B * A           # 12
CHUNK = 512             # Sq chunk (one PSUM bank of f32)


def build_nc(n_tasks=NT):
    f32 = mybir.dt.float32
    bf16 = mybir.dt.bfloat16
    ADD = mybir.AluOpType.add
    EXP = mybir.ActivationFunctionType.Exp

    nc = bacc.Bacc(None, target_bir_lowering=False, debug=False)
    qT_d = nc.declare_dram_parameter("qT", [n_tasks, E, S], bf16, isOutput=False)
    kT_d = nc.declare_dram_parameter("kT", [n_tasks, E, S], bf16, isOutput=False)
    vT_d = nc.declare_dram_parameter("vT", [n_tasks, E, S], bf16, isOutput=False)
    wq_d = nc.declare_dram_parameter("wq", [n_tasks, E, F], bf16, isOutput=False)
    wk_d = nc.declare_dram_parameter("wk", [n_tasks, E, F], bf16, isOutput=False)
    wv_d = nc.declare_dram_parameter("wv", [n_tasks, E, F], bf16, isOutput=False)
    wo_d = nc.declare_dram_parameter("wo", [n_tasks, 4, DK, OUTD], bf16, isOutput=False)
    bq_d = nc.declare_dram_parameter("bq", [n_tasks, F], f32, isOutput=False)
    bk_d = nc.declare_dram_parameter("bk", [n_tasks, F], f32, isOutput=False)
    bv_d = nc.declare_dram_parameter("bv", [n_tasks, F], f32, isOutput=False)
    out_d = nc.declare_dram_parameter("out", [n_tasks, 4, 2, P, S], bf16, isOutput=True)
    den_d = nc.declare_dram_parameter("den", [n_tasks, 4, S], bf16, isOutput=True)

    with TileContext(nc) as tc, ExitStack() as ctx:
        inbuf = ctx.enter_context(tc.tile_pool(name="inbuf", bufs=2))
        wbuf = ctx.enter_context(tc.tile_pool(name="wbuf", bufs=2))
        proj = ctx.enter_context(tc.tile_pool(name="proj", bufs=2))
        ptbuf = ctx.enter_context(tc.tile_pool(name="ptbuf", bufs=6))
        xnbuf = ctx.enter_context(tc.tile_pool(name="xnbuf", bufs=2))
        obuf = ctx.enter_context(tc.tile_pool(name="obuf", bufs=3))
        psS = ctx.enter_context(tc.tile_pool(name="psS", bufs=2, space="PSUM"))
        psX = ctx.enter_context(tc.tile_pool(name="psX", bufs=1, space="PSUM"))
        psP = ctx.enter_context(tc.tile_pool(name="psP", bufs=2, space="PSUM"))

        def load_task(t, spread=False, parts="ab"):
            """Allocate task t's input tiles and issue the part-a DMAs
            (QK-projection inputs).  Part b (V inputs + Wo) is issued
            later via load_task_b so the issue bursts are small and the
            transfers can't crowd out the critical path.  spread=True
            (cold start) fans everything over three engine queues."""
            qT_sb = inbuf.tile([P, 2, S], bf16, tag="qT", name="qT_sb")
            kT_sb = inbuf.tile([P, 2, S], bf16, tag="kT", name="kT_sb")
            vT_sb = inbuf.tile([P, 2, S], bf16, tag="vT", name="vT_sb")
            wq_sb = wbuf.tile([P, 2, F], bf16, tag="wq", name="wq_sb")
            wk_sb = wbuf.tile([P, 2, F], bf16, tag="wk", name="wk_sb")
            wv_sb = wbuf.tile([P, 2, F], bf16, tag="wv", name="wv_sb")
            wo_sb = wbuf.tile([DK, 4, OUTD], bf16, tag="wo", name="wo_sb")
            bq_sb = wbuf.tile([P, 2], f32, tag="bq", name="bq_sb")
            bk_sb = wbuf.tile([P, 2], f32, tag="bk", name="bk_sb")
            bv_bc = wbuf.tile([P, F], f32, tag="bvbc", name="bv_bc")
            vsb = proj.tile([P, 8, 4, DK + 1], bf16, tag="vsb", name="vsb")
            nc.gpsimd.memset(vsb[:, :, :, DK : DK + 1], 1.0)
            ld = (qT_sb, kT_sb, vT_sb, wq_sb, wk_sb, wv_sb, wo_sb,
                  bq_sb, bk_sb, bv_bc, vsb)
            queues = [nc.sync, nc.scalar, nc.gpsimd] if spread else [nc.sync, nc.gpsimd]
            if "a" in parts:
                _issue_ld_a(t, ld, queues)
            if "b" in parts:
                _issue_ld_b(t, ld, queues)
            return ld

        def _issue_ld_a(t, ld, queues):
            qT_sb, kT_sb, vT_sb, wq_sb, wk_sb, wv_sb, wo_sb, bq_sb, bk_sb, bv_bc, vsb = ld
            qTr = qT_d[t].rearrange("(e p) s -> p e s", p=P)
            kTr = kT_d[t].rearrange("(e p) s -> p e s", p=P)
            xfers = [
                (wq_sb, wq_d[t].rearrange("(e p) f -> p e f", p=P)),
                (wk_sb, wk_d[t].rearrange("(e p) f -> p e f", p=P)),
                (qT_sb[:, 0:1, :], qTr[:, 0:1, :]),
                (qT_sb[:, 1:2, :], qTr[:, 1:2, :]),
                (kT_sb[:, 0:1, :], kTr[:, 0:1, :]),
                (kT_sb[:, 1:2, :], kTr[:, 1:2, :]),
                (bq_sb, bq_d[t].rearrange("(e p) -> p e", p=P)),
                (bk_sb, bk_d[t].rearrange("(e p) -> p e", p=P)),
            ]
            for i, (dst, src) in enumerate(xfers):
                queues[i % len(queues)].dma_start(out=dst, in_=src)

        def _issue_ld_b(t, ld, queues):
            qT_sb, kT_sb, vT_sb, wq_sb, wk_sb, wv_sb, wo_sb, bq_sb, bk_sb, bv_bc, vsb = ld
            vTr = vT_d[t].rearrange("(e p) s -> p e s", p=P)
            xfers = [
                (wv_sb, wv_d[t].rearrange("(e p) f -> p e f", p=P)),
                (vT_sb[:, 0:1, :], vTr[:, 0:1, :]),
                (vT_sb[:, 1:2, :], vTr[:, 1:2, :]),
                (bv_bc, bv_d[t].partition_broadcast(P)),
                (wo_sb, wo_d[t].rearrange("h p m -> p h m")),
            ]
            for i, (dst, src) in enumerate(xfers):
                queues[i % len(queues)].dma_start(out=dst, in_=src)

        def qk_unit(ld, dsts, di, eo, n):
            """One Q-or-K projection unit: both contraction halves into one
            PSUM bank, bias-add evacuation on DVE."""
            qT_sb, kT_sb, vT_sb, wq_sb, wk_sb, wv_sb, wo_sb, bq_sb, bk_sb, bv_bc, vsb = ld
            dst, srct, w_sb, b_sb = (
                (dsts[0], qT_sb, wq_sb, bq_sb),
                (dsts[1], kT_sb, wk_sb, bk_sb),
            )[di]
            ps = psP.tile([P, CHUNK], f32, tag="psp", name="psqk")
            for ek in range(2):
                nc.tensor.matmul(
                    ps,
                    lhsT=w_sb[:, ek, 128 * eo : 128 * eo + 128],
                    rhs=srct[:, ek, CHUNK * n : CHUNK * (n + 1)],
                    start=(ek == 0),
                    stop=(ek == 1),
                )
            nc.vector.tensor_tensor(
                out=dst[:, eo, CHUNK * n : CHUNK * (n + 1)],
                in0=ps,
                in1=b_sb[:, eo : eo + 1].to_broadcast((P, CHUNK)),
                op=ADD,
            )

        def v_unit(ld, m):
            """V projection for one S tile (m): both contraction halves,
            bias-add evacuation on Pool."""
            qT_sb, kT_sb, vT_sb, wq_sb, wk_sb, wv_sb, wo_sb, bq_sb, bk_sb, bv_bc, vsb = ld
            psv = psP.tile([P, F], f32, tag="psp", name="psv")
            for ek in range(2):
                nc.tensor.matmul(
                    psv,
                    lhsT=vT_sb[:, ek, 128 * m : 128 * m + 128],
                    rhs=wv_sb[:, ek, :],
                    start=(ek == 0),
                    stop=(ek == 1),
                )
            nc.vector.tensor_tensor(
                out=vsb[:, m, :, 0:DK],
                in0=psv.rearrange("p (h d) -> p h d", h=4),
                in1=bv_bc.rearrange("p (h d) -> p h d", h=4),
                op=ADD,
            )

        op_ctr = [0]

        def op_unit(t, h, xsb, wo_sb, k):
            """One output-projection matmul (RAW, unnormalized) + ship."""
            mo, n = k // 2, k % 2
            pso = psP.tile([P, CHUNK], f32, tag="psp", name="pso")
            nc.tensor.matmul(
                pso,
                lhsT=wo_sb[0:DK, h, 128 * mo : 128 * mo + 128],
                rhs=xsb[0:DK, n, :],
                start=True,
                stop=True,
            )
            osb = obuf.tile([P, CHUNK], bf16, tag="osb", name="osb")
            j = op_ctr[0]
            op_ctr[0] += 1
            nc.vector.tensor_copy(out=osb, in_=pso)
            q = nc.sync if j % 2 == 0 else nc.gpsimd
            q.dma_start(
                out=out_d[t, h, mo, :, CHUNK * n : CHUNK * (n + 1)], in_=osb
            )

        def evac_head(t, h, psx, tail=False):
            """Evacuate head h's AV accumulator (+denominator row) to SBUF
            in one wide DVE op (ACT stays exp-only), ship the denominators."""
            xsb = xnbuf.tile([P, 2, CHUNK], bf16, tag="xsb", name="xsb")
            if tail:
                nc.scalar.activation(
                    out=xsb[0 : DK + 1, 0, :],
                    in_=psx[0 : DK + 1, 0:CHUNK],
                    func=mybir.ActivationFunctionType.Copy,
                )
                nc.vector.tensor_copy(
                    out=xsb[0 : DK + 1, 1, :],
                    in_=psx[0 : DK + 1, CHUNK : 2 * CHUNK],
                )
            else:
                nc.vector.tensor_copy(
                    out=xsb[0 : DK + 1, :, :],
                    in_=psx[0 : DK + 1, :].rearrange("p (a c) -> p a c", a=2),
                )
            nc.sync.dma_start(out=den_d[t, h], in_=xsb[DK : DK + 1, :, :])
            return xsb

        # Warm the PE p-state during the initial input-DMA wait; the dummy
        # exp pre-loads the ACT function table.
        warm = wbuf.tile([P, P], bf16, tag="warm", name="warm")
        nc.gpsimd.memset(warm, 0.0)
        warmo = wbuf.tile([1, 32], bf16, tag="warmo", name="warmo")
        nc.scalar.activation(out=warmo, in_=warm[0:1, 0:32], func=EXP, scale=0.125)
        for w in range(16):
            psw = psP.tile([P, 64], f32, tag="psp", name="psw")
            nc.tensor.matmul(psw, lhsT=warm, rhs=warm[:, 0:64], start=True, stop=True)

        # ---- cold start: task 0's own projections (eo=0 heads + V head) ----
        ld = load_task(0, spread=True)
        qproj = proj.tile([P, 2, S], bf16, tag="qproj", name="qproj")
        kproj = proj.tile([P, 2, S], bf16, tag="kproj", name="kproj")
        for di in range(2):
            for n in range(2):
                qk_unit(ld, (qproj, kproj), di, 0, n)

        state = (qproj, kproj, ld, ld[10], ld[6])
        ld_next = qproj_n = kproj_n = None
        from collections import deque

        backlog = deque()    # pending AV pairs: (vsb, pt, h, m, t, wo)
        av_psx = [None]      # current head's AV accumulator
        done_heads = []      # heads whose AV finished this slot
        pending_op = None    # [t, h, xsb, wo, next_unit]

        def emit_av(entry):
            evsb, ept, eh, em, et, ewo = entry
            if em == 0:
                av_psx[0] = psX.tile([P, 2 * CHUNK], f32, tag="psx", name="psx")
            for n in range(2):
                nc.tensor.matmul(
                    av_psx[0][0 : DK + 1, CHUNK * n : CHUNK * (n + 1)],
                    lhsT=evsb[:, em, eh, :],
                    rhs=ept[:, CHUNK * n : CHUNK * (n + 1)],
                    start=(em == 0),
                    stop=(em == 7),
                )
            if em == 7:
                done_heads.append((et, eh, av_psx[0], ewo))

        def backlog_target(t, h, m):
            # AV pairs run 2 slots behind their exp so the exp's ~1.1us
            # latency never sits on the PE critical path (at depth 1 the
            # AV wait alternated the cadence up to ~1.2us/slot).  Task 0
            # head 0 runs 4 behind so its AVs also never wait on the
            # still-streaming vT input; taper back at h1.
            if t == 0 and h == 0:
                return 4
            if t == 0 and h == 1 and m == 0:
                return 3
            return 2

        for t in range(n_tasks):
            qproj, kproj, ld, vsb_cur, wo_sb = state
            last = t + 1 >= n_tasks
            for h in range(4):
                e, r0 = h // 2, 64 * (h % 2)
                for m in range(8):
                    # ---- scores pair + exp ------------------------------
                    pss = psS.tile([P, 2 * CHUNK], f32, tag="pss", name="pss")
                    for n in range(2):
                        nc.tensor.matmul(
                            pss[:, CHUNK * n : CHUNK * (n + 1)],
                            lhsT=kproj[r0 : r0 + 64, e, 128 * m : 128 * m + 128],
                            rhs=qproj[r0 : r0 + 64, e, CHUNK * n : CHUNK * (n + 1)],
                            start=True,
                            stop=True,
                        )
                    pt = ptbuf.tile([P, 2 * CHUNK], bf16, tag="pt", name="pt")
                    nc.scalar.activation(out=pt, in_=pss, func=EXP, scale=0.125)
                    # ---- AV pairs (software-pipelined via backlog) ------
                    backlog.append((vsb_cur, pt, h, m, t, wo_sb))
                    while len(backlog) > backlog_target(t, h, m):
                        emit_av(backlog.popleft())
                    # ---- output-projection units on even slots (odd
                    # slots carry qk units, interleaving their DVE
                    # evacuations so the PSUM ring never waits long) -----
                    if pending_op is not None and m % 2 == 0:
                        ot, oh, oxsb, owo, k = pending_op
                        op_unit(ot, oh, oxsb, owo, k)
                        pending_op = None if k == 3 else [ot, oh, oxsb, owo, k + 1]
                    # next-task load issue, two small bursts: part a (QK
                    # inputs) then part b (V inputs + Wo).  Emitted after
                    # PE-dependent DMAs (den / output shipments) on the
                    # same queues, so the engines cannot race ahead and
                    # the transfers can't crowd the current task's inputs
                    lda_h, ldb_h = (1, 2) if t == 0 else (0, 1)
                    if not last and h == lda_h and m == 5:
                        ld_next = load_task(t + 1, parts="a")
                        qproj_n = proj.tile([P, 2, S], bf16, tag="qproj", name="qproj")
                        kproj_n = proj.tile([P, 2, S], bf16, tag="kproj", name="kproj")
                    if not last and h == ldb_h and m == 2:
                        _issue_ld_b(t + 1, ld_next, [nc.sync, nc.gpsimd])
                    # every task runs its own V projections in its h0 slots,
                    # ahead of the AV consumers
                    if h == 0:
                        v_unit(ld, m)
                    if t == 0 and h == 1 and m in (0, 3, 5, 7):
                        di, n = ((0, 0), (1, 0), (0, 1), (1, 1))[
                            (0, 3, 5, 7).index(m)
                        ]
                        qk_unit(ld, (qproj, kproj), di, 1, n)
                    if not last:
                        # next-task QK projection units (odd slots)
                        qk_slots = (
                            (1, 5), (1, 7), (2, 3), (2, 5),
                            (2, 7), (3, 3), (3, 5), (3, 7),
                        ) if t > 0 else (
                            (2, 3), (2, 5), (2, 7), (3, 1),
                            (3, 0), (3, 3), (3, 5), (3, 7),
                        )
                        if (h, m) in qk_slots:
                            u = qk_slots.index((h, m))
                            qk_unit(
                                ld_next, (qproj_n, kproj_n),
                                di=(u // 2) % 2, eo=u // 4, n=u % 2,
                            )
                    # ---- head-completion evacuations (after foreign, so
                    # the foreign unit's PSUM ring isn't stuck behind the
                    # wide xsb copy on the DVE queue) --------------------
                    for et, eh, epsx, ewo in done_heads:
                        xsb = evac_head(et, eh, epsx)
                        pending_op = [et, eh, xsb, ewo, 0]
                    done_heads.clear()
            if not last:
                state = (qproj_n, kproj_n, ld_next, ld_next[10], ld_next[6])

        # ---- tail: last head's AVs + evac + output projection -----------
        if pending_op is not None:
            ot, oh, oxsb, owo, k = pending_op
            for kk in range(k, 4):
                op_unit(ot, oh, oxsb, owo, kk)
        while backlog:
            emit_av(backlog.popleft())
        (dt_, dh, dpsx, dwo) = done_heads.pop()
        wo_sb = dwo
        xsb = evac_head(dt_, dh, dpsx, tail=True)
        for mo in range(2):
            # scores pool is idle at the tail; its 2-bank tiles host both
            # chunks so the matmuls never wait on an evacuation
            ps2 = psS.tile([P, 2 * CHUNK], f32, tag="pss", name="pso2")
            for n in range(2):
                nc.tensor.matmul(
                    ps2[:, CHUNK * n : CHUNK * (n + 1)],
                    lhsT=wo_sb[0:DK, dh, 128 * mo : 128 * mo + 128],
                    rhs=xsb[0:DK, n, :],
                    start=True,
                    stop=True,
                )
            osb = obuf.tile([P, 2, CHUNK], bf16, tag="osbt", name="osbt")
            for n in range(2):
                if n == 0:
                    nc.scalar.activation(
                        out=osb[:, n, :],
                        in_=ps2[:, 0:CHUNK],
                        func=mybir.ActivationFunctionType.Copy,
                    )
                else:
                    nc.vector.tensor_copy(
                        out=osb[:, n, :], in_=ps2[:, CHUNK : 2 * CHUNK]
                    )
                q = nc.sync if n == 0 else nc.gpsimd
                q.dma_start(
                    out=out_d[dt_, dh, mo, :, CHUNK * n : CHUNK * (n + 1)],
                    in_=osb[:, n, :],
                )

    nc.finalize()
    return nc


_cache = {}


def _get_nc():
    if "nc" not in _cache:
        _cache["nc"] = build_nc()
    return _cache["nc"]


def _tasks_of(c):
    return [NT * c + j for j in range(NT)]


def make_in_maps(query, key, value, Wq, bq, Wk, bk, Wv, bv, Wo, bo):
    import ml_dtypes

    in_dt = ml_dtypes.bfloat16
    f = np.float32
    q = np.asarray(query, f).reshape(NPAIR, S, E)
    k = np.asarray(key, f).reshape(NPAIR, S, E)
    v = np.asarray(value, f).reshape(NPAIR, S, E)
    qT = np.ascontiguousarray(q.transpose(0, 2, 1))
    kT = np.ascontiguousarray(k.transpose(0, 2, 1))
    vT = np.ascontiguousarray(v.transpose(0, 2, 1))
    Wq_, Wk_, Wv_, Wo_ = (np.asarray(w, f) for w in (Wq, Wk, Wv, Wo))
    bq_, bk_, bv_ = (np.asarray(b, f) for b in (bq, bk, bv))

    in_maps = []
    for c in range(NCORES):
        ts = _tasks_of(c)
        pairs = [t // 2 for t in ts]
        sls = [slice(F * (t % 2), F * (t % 2) + F) for t in ts]
        in_maps.append(
            {
                "qT": np.ascontiguousarray(qT[pairs]).astype(in_dt),
                "kT": np.ascontiguousarray(kT[pairs]).astype(in_dt),
                "vT": np.ascontiguousarray(vT[pairs]).astype(in_dt),
                "wq": np.ascontiguousarray(np.stack([Wq_[:, s] for s in sls])).astype(in_dt),
                "wk": np.ascontiguousarray(np.stack([Wk_[:, s] for s in sls])).astype(in_dt),
                "wv": np.ascontiguousarray(np.stack([Wv_[:, s] for s in sls])).astype(in_dt),
                "wo": np.ascontiguousarray(np.stack([Wo_[s, :].reshape(4, DK, OUTD) for s in sls])).astype(in_dt),
                "bq": np.stack([bq_[s] for s in sls]),
                "bk": np.stack([bk_[s] for s in sls]),
                "bv": np.stack([bv_[s] for s in sls]),
            }
        )
    return in_maps


def assemble_output(results, bo):
    out = np.zeros((NPAIR, S, OUTD), np.float32)
    for c in range(NCORES):
        o = np.asarray(results[c]["out"], np.float32)     # [NT, 4, 2, 128, S]
        den = np.asarray(results[c]["den"], np.float32)   # [NT, 4, S]
        for j, t in enumerate(_tasks_of(c)):
            x = o[j].reshape(4, OUTD, S) / den[j][:, None, :]
            out[t // 2] += x.sum(0).T
    out += np.asarray(bo, np.float32)
    return out.reshape(B, A, S, OUTD)


def kernel(query, key, value, Wq, bq, Wk, bk, Wv, bv, Wo, bo):
    import time

    in_maps = make_in_maps(query, key, value, Wq, bq, Wk, bk, Wv, bv, Wo, bo)
    last_err = None
    for _ in range(3):  # the device occasionally reports a transient
        try:            # NRT_EXEC_UNIT_UNRECOVERABLE on a fresh load; retry
            res = run_bass_kernel_spmd(
                _get_nc(), in_maps, core_ids=list(range(NCORES))
            )
            out = assemble_output(res.results, bo)
            if np.isfinite(out).all():
                return out
            last_err = RuntimeError("non-finite output")
        except Exception as e:  # noqa: BLE001
            last_err = e
        time.sleep(2)
    raise last_err
